# revision 3
# baseline (speedup 1.0000x reference)
"""AdaptiveEMA Trainium2 kernel (8 NeuronCores, batch-sharded).

Reference computes, per channel c=(b,f) and time t:
    out[c,t] = sum_{k=0..K-1, k<=t} alpha_c^k * x[c,t-k] / (csum_c[min(t,K-1)] + 1e-8)
with K=201, alpha_c = 0.5**(1/exp(log_halflife[f])), csum_c[j] = sum_{i<=j} alpha_c^i.

We use the equivalent first-order recurrence
    y[t] = alpha*y[t-1] + (x[t] - alpha^K * x[t-K])        (x[<0] = 0)
    out[t] = y[t] * inv_weight[min(t,K-1)]
which maps to one scalar_tensor_tensor (u) + one tensor_tensor_scan (y) +
two normalization multiplies per [128, 4096] tile. alpha^K, and the
reciprocal weight tables are precomputed on host from the tiny [F] parameter.

Sharding: batch dim across 8 cores (4 batches/core -> 1024 channels/core,
8 partition-tiles of [128 x 4096]). No cross-core communication.
"""

import math
from contextlib import ExitStack

import numpy as np

import concourse.bass as bass
import concourse.mybir as mybir
import concourse.tile as tile
from concourse import bacc
from concourse.bass_utils import run_bass_kernel_spmd

B, F, S = 32, 256, 4096
MAX_SIZE = 200
K = MAX_SIZE + 1          # 201 taps
N_CORES = 8
B_LOC = B // N_CORES      # 4 batches per core
C = B_LOC * F             # 1024 channels per core
P = 128
NT = C // P               # 8 partition tiles per core
NPAR = F // P             # 2 distinct parameter column groups (tile parity)

F32 = mybir.dt.float32
OP_MULT = mybir.AluOpType.mult
OP_ADD = mybir.AluOpType.add


def build_bass():
    nc = bacc.Bacc("TRN2", target_bir_lowering=False, debug=False, num_devices=N_CORES)

    x = nc.declare_dram_parameter("x", [C, S], F32, isOutput=False)
    alpha2 = nc.declare_dram_parameter("alpha2", [P, NPAR], F32, isOutput=False)
    negak2 = nc.declare_dram_parameter("negak2", [P, NPAR], F32, isOutput=False)
    invc2 = nc.declare_dram_parameter("invc2", [P, NPAR], F32, isOutput=False)
    invt2 = nc.declare_dram_parameter("invt2", [P, NPAR * MAX_SIZE], F32, isOutput=False)
    out = nc.declare_dram_parameter("out", [C, S], F32, isOutput=True)

    with ExitStack() as ctx:
        tc = ctx.enter_context(tile.TileContext(nc))
        const_pool = ctx.enter_context(tc.tile_pool(name="const", bufs=1))
        xpool = ctx.enter_context(tc.tile_pool(name="xp", bufs=3))
        ypool = ctx.enter_context(tc.tile_pool(name="yp", bufs=3))

        alpha_sb = const_pool.tile([P, NPAR], F32)
        nc.sync.dma_start(alpha_sb[:], alpha2[:])
        negak_sb = const_pool.tile([P, NPAR], F32)
        nc.sync.dma_start(negak_sb[:], negak2[:])
        invc_sb = const_pool.tile([P, NPAR], F32)
        nc.sync.dma_start(invc_sb[:], invc2[:])
        invt_sb = const_pool.tile([P, NPAR * MAX_SIZE], F32)
        nc.sync.dma_start(invt_sb[:], invt2[:])

        for j in range(NT):
            p = j % NPAR
            xt = xpool.tile([P, K + S], F32)
            nc.gpsimd.memzero(xt[:, 0:K])
            nc.sync.dma_start(xt[:, K:K + S], x[j * P:(j + 1) * P, :])

            yt = ypool.tile([P, S], F32)
            # u[t] = x[t] - alpha^K * x[t-K]  (valid for all t with zero pad)
            nc.vector.scalar_tensor_tensor(
                out=yt[:],
                in0=xt[:, 0:S],
                scalar=negak_sb[:, p:p + 1],
                in1=xt[:, K:K + S],
                op0=OP_MULT,
                op1=OP_ADD,
            )
            # y[t] = alpha * y[t-1] + u[t]
            nc.vector.tensor_tensor_scan(
                out=yt[:],
                data0=alpha_sb[:, p:p + 1].broadcast_to([P, S]),
                data1=yt[:],
                initial=0.0,
                op0=OP_MULT,
                op1=OP_ADD,
            )
            # steady-state normalization (t >= MAX_SIZE): constant weight
            nc.scalar.mul(yt[:, MAX_SIZE:S], yt[:, MAX_SIZE:S], invc_sb[:, p:p + 1])
            # ramp-up region (t < MAX_SIZE): per-t weight table
            nc.gpsimd.tensor_mul(
                yt[:, 0:MAX_SIZE],
                yt[:, 0:MAX_SIZE],
                invt_sb[:, p * MAX_SIZE:(p + 1) * MAX_SIZE],
            )
            nc.sync.dma_start(out[j * P:(j + 1) * P, :], yt[:])

    nc.finalize()
    return nc


_NC_CACHE = None


def _get_nc():
    global _NC_CACHE
    if _NC_CACHE is None:
        _NC_CACHE = build_bass()
    return _NC_CACHE


def _host_params(log_halflife):
    """Precompute per-feature scan/normalization constants (float64 host math)."""
    lh = log_halflife.astype(np.float64)
    alpha = 0.5 ** (1.0 / np.exp(lh))                     # [F]
    aK = alpha ** K                                        # [F]
    # csum[f, j] = sum_{i<=j} alpha^i   for j in [0, K)
    powers = alpha[:, None] ** np.arange(K, dtype=np.float64)[None, :]
    csum = np.cumsum(powers, axis=1)                       # [F, K]
    inv_all = 1.0 / (csum + 1e-8)                          # [F, K]

    def fold(v):  # [F, ...] -> [P, NPAR, ...] column-grouped, f = 128*p + lane
        return np.ascontiguousarray(
            v.reshape(NPAR, P, *v.shape[1:]).swapaxes(0, 1)
        )

    alpha2 = fold(alpha).astype(np.float32)                       # [P, NPAR]
    negak2 = fold(-aK).astype(np.float32)                         # [P, NPAR]
    invc2 = fold(inv_all[:, MAX_SIZE]).astype(np.float32)         # [P, NPAR]
    invt2 = (
        fold(inv_all[:, :MAX_SIZE]).reshape(P, NPAR * MAX_SIZE).astype(np.float32)
    )                                                             # [P, NPAR*200]
    return alpha2, negak2, invc2, invt2


def run(x, log_halflife, trace=False):
    x = np.ascontiguousarray(np.asarray(x, dtype=np.float32))
    log_halflife = np.asarray(log_halflife, dtype=np.float32)
    assert x.shape == (B, F, S) and log_halflife.shape == (F,)

    alpha2, negak2, invc2, invt2 = _host_params(log_halflife)
    in_maps = []
    for i in range(N_CORES):
        shard = np.ascontiguousarray(
            x[i * B_LOC:(i + 1) * B_LOC].reshape(C, S)
        )
        in_maps.append({
            "x": shard,
            "alpha2": alpha2,
            "negak2": negak2,
            "invc2": invc2,
            "invt2": invt2,
        })

    nc = _get_nc()
    res = run_bass_kernel_spmd(nc, in_maps, core_ids=list(range(N_CORES)), trace=trace)
    outs = [res.results[i]["out"].reshape(B_LOC, F, S) for i in range(N_CORES)]
    full = np.concatenate(outs, axis=0)
    return full, res.exec_time_ns


def kernel(x, log_halflife):
    out, _ = run(x, log_halflife, trace=False)
    return out


# revision 5
# speedup vs baseline: 1.0293x; 1.0293x over previous
"""AdaptiveEMA Trainium2 kernel (8 NeuronCores, batch-sharded).

Reference, per channel c=(b,f) and time t (K=201, alpha_c from log_halflife[f]):
    out[c,t] = sum_{k=0..K-1, k<=t} alpha_c^k * x[c,t-k] / (csum_c[min(t,K-1)] + 1e-8)

Math used here: with yf = scan(alpha, x) (untruncated EMA, exact identity)
    y[t] = yf[t] - alpha^K * yf[t-K]
so per [128 x 4096] tile:
    yf   = tensor_tensor_scan(alpha, x)              (DVE; fp32 state, fp16 out)
    out[t<200]   = yf[t] * invt[t]                   (DVE ramp multiply)
    out[t>=200]  = invc*yf[t] - aK*invc*yf[t-K]      (ScalarE scale-copies +
                                                      GpSimd/DVE add, split SPLIT)
alpha^K and the reciprocal-weight tables are precomputed on host from the tiny
[F] parameter. Sharding: batch dim across 8 cores (4 batches/core -> 1024
channels/core); no cross-device communication.

The scan keeps fp32 state internally regardless of operand dtype, so the fp16
intermediates only add per-element rounding (~5e-4 rel), no accumulation.
"""

import math
from contextlib import ExitStack

import numpy as np

import concourse.bass as bass
import concourse.mybir as mybir
import concourse.tile as tile
from concourse import bacc
from concourse.bass_utils import run_bass_kernel_spmd

B, F, S = 32, 256, 4096
MAX_SIZE = 200
K = MAX_SIZE + 1          # 201 taps
N_CORES = 8
B_LOC = B // N_CORES      # 4 batches per core
C = B_LOC * F             # 1024 channels per core
P = 128
NT = C // P               # 8 partition tiles per core
NPAR = F // P             # 2 distinct parameter column groups (tile parity)
NS = S - MAX_SIZE         # steady-region columns (3896)

F32 = mybir.dt.float32
F16 = mybir.dt.float16
OP_MULT = mybir.AluOpType.mult
OP_ADD = mybir.AluOpType.add

# tiles whose steady-region add runs on GpSimd (rest: fused STT on DVE)
GPSIMD_TILES = frozenset((0, 2, 4, 6))


def build_bass():
    nc = bacc.Bacc("TRN2", target_bir_lowering=False, debug=False, num_devices=N_CORES)

    x = nc.declare_dram_parameter("x", [C, S], F32, isOutput=False)
    alpha2 = nc.declare_dram_parameter("alpha2", [P, NPAR], F32, isOutput=False)
    nakinv2 = nc.declare_dram_parameter("nakinv2", [P, NPAR], F32, isOutput=False)
    invc2 = nc.declare_dram_parameter("invc2", [P, NPAR], F32, isOutput=False)
    invt2 = nc.declare_dram_parameter("invt2", [P, NPAR * MAX_SIZE], F32, isOutput=False)
    out = nc.declare_dram_parameter("out", [C, S], F16, isOutput=True)

    with ExitStack() as ctx:
        tc = ctx.enter_context(tile.TileContext(nc))
        const_pool = ctx.enter_context(tc.tile_pool(name="const", bufs=1))
        xpool = ctx.enter_context(tc.tile_pool(name="xp", bufs=3))
        ypool = ctx.enter_context(tc.tile_pool(name="yp", bufs=3))
        opool = ctx.enter_context(tc.tile_pool(name="op", bufs=3))
        bpool = ctx.enter_context(tc.tile_pool(name="bp", bufs=2))

        alpha_sb = const_pool.tile([P, NPAR], F32)
        nc.sync.dma_start(alpha_sb[:], alpha2[:])
        nakinv_sb = const_pool.tile([P, NPAR], F32)
        nc.sync.dma_start(nakinv_sb[:], nakinv2[:])
        invc_sb = const_pool.tile([P, NPAR], F32)
        nc.sync.dma_start(invc_sb[:], invc2[:])
        invt_sb = const_pool.tile([P, NPAR * MAX_SIZE], F32)
        nc.sync.dma_start(invt_sb[:], invt2[:])

        for j in range(NT):
            p = j % NPAR
            xt = xpool.tile([P, S], F32)
            nc.sync.dma_start(xt[:], x[j * P:(j + 1) * P, :])

            # yf[t] at column t+1; column 0 is zero (yf[-1]) so the shifted
            # read below stays 4B-aligned for fp16 packed modes.
            yft = ypool.tile([P, S + 1], F16)
            nc.gpsimd._memset_packed(yft[:, 0:1], 0)
            nc.vector.tensor_tensor_scan(
                out=yft[:, 1:S + 1],
                data0=alpha_sb[:, p:p + 1].broadcast_to([P, S]),
                data1=xt[:],
                initial=0.0,
                op0=OP_MULT,
                op1=OP_ADD,
            )

            ot = opool.tile([P, S], F16)
            # ramp region t in [0, 200): out = yf[t] * invt[t]
            nc.vector.tensor_mul(
                ot[:, 0:MAX_SIZE],
                yft[:, 1:MAX_SIZE + 1],
                invt_sb[:, p * MAX_SIZE:(p + 1) * MAX_SIZE],
            )
            # steady main term t in [200, S): out = invc * yf[t]
            nc.scalar.mul(ot[:, MAX_SIZE:S], yft[:, MAX_SIZE + 1:S + 1],
                          invc_sb[:, p:p + 1])
            # correction: out[t] -= aK*invc * yf[t-K]  (t-K+1 = t-200 in yft cols,
            # with yft[:,0]=0 covering t=200)
            if j in GPSIMD_TILES:
                bt = bpool.tile([P, NS], F16)
                nc.scalar.mul(bt[:], yft[:, 0:NS], nakinv_sb[:, p:p + 1])
                nc.gpsimd.tensor_add(ot[:, MAX_SIZE:S], ot[:, MAX_SIZE:S], bt[:])
            else:
                nc.vector.scalar_tensor_tensor(
                    out=ot[:, MAX_SIZE:S],
                    in0=yft[:, 0:NS],
                    scalar=nakinv_sb[:, p:p + 1],
                    in1=ot[:, MAX_SIZE:S],
                    op0=OP_MULT,
                    op1=OP_ADD,
                )
            nc.sync.dma_start(out[j * P:(j + 1) * P, :], ot[:])

    nc.finalize()
    return nc


_NC_CACHE = None


def _get_nc():
    global _NC_CACHE
    if _NC_CACHE is None:
        _NC_CACHE = build_bass()
    return _NC_CACHE


def _host_params(log_halflife):
    """Precompute per-feature scan/normalization constants (float64 host math)."""
    lh = log_halflife.astype(np.float64)
    alpha = 0.5 ** (1.0 / np.exp(lh))                     # [F]
    aK = alpha ** K                                        # [F]
    powers = alpha[:, None] ** np.arange(K, dtype=np.float64)[None, :]
    csum = np.cumsum(powers, axis=1)                       # [F, K]
    inv_all = 1.0 / (csum + 1e-8)                          # [F, K]
    invc = inv_all[:, MAX_SIZE]                            # [F]

    def fold(v):  # [F, ...] -> [P, NPAR, ...] column-grouped, f = 128*p + lane
        return np.ascontiguousarray(
            v.reshape(NPAR, P, *v.shape[1:]).swapaxes(0, 1)
        )

    alpha2 = fold(alpha).astype(np.float32)                       # [P, NPAR]
    nakinv2 = fold(-aK * invc).astype(np.float32)                 # [P, NPAR]
    invc2 = fold(invc).astype(np.float32)                         # [P, NPAR]
    invt2 = (
        fold(inv_all[:, :MAX_SIZE]).reshape(P, NPAR * MAX_SIZE).astype(np.float32)
    )                                                             # [P, NPAR*200]
    return alpha2, nakinv2, invc2, invt2


def run(x, log_halflife, trace=False):
    x = np.ascontiguousarray(np.asarray(x, dtype=np.float32))
    log_halflife = np.asarray(log_halflife, dtype=np.float32)
    assert x.shape == (B, F, S) and log_halflife.shape == (F,)

    alpha2, nakinv2, invc2, invt2 = _host_params(log_halflife)
    in_maps = []
    for i in range(N_CORES):
        shard = np.ascontiguousarray(
            x[i * B_LOC:(i + 1) * B_LOC].reshape(C, S)
        )
        in_maps.append({
            "x": shard,
            "alpha2": alpha2,
            "nakinv2": nakinv2,
            "invc2": invc2,
            "invt2": invt2,
        })

    nc = _get_nc()
    res = run_bass_kernel_spmd(nc, in_maps, core_ids=list(range(N_CORES)), trace=trace)
    full = np.empty((B, F, S), dtype=np.float32)
    for i in range(N_CORES):
        full[i * B_LOC:(i + 1) * B_LOC] = (
            res.results[i]["out"].astype(np.float32).reshape(B_LOC, F, S)
        )
    return full, res.exec_time_ns


def kernel(x, log_halflife):
    out, _ = run(x, log_halflife, trace=False)
    return out


# revision 6
# speedup vs baseline: 1.3024x; 1.2653x over previous
"""AdaptiveEMA Trainium2 kernel (8 NeuronCores, batch-sharded).

Reference, per channel c=(b,f) and time t (K=201, alpha_c from log_halflife[f]):
    out[c,t] = sum_{k=0..K-1, k<=t} alpha_c^k * x[c,t-k] / (csum_c[min(t,K-1)] + 1e-8)

Math used here: with yf = scan(alpha, x) (untruncated EMA) the truncated sum is
exactly y[t] = yf[t] - alpha^K * yf[t-K], so per [128 x 4096] tile:
    yf   = tensor_tensor_scan(alpha, x)            (DVE; fp32 state, fp16 out)
    out[t<200]  = yf[t] * invt[t]                  (DVE ramp multiply, 200 cols)
    out[t>=200] = invc*yf[t] - aK*invc*yf[t-K]     (TensorE: two accumulating
        matmuls per 487-col chunk with per-channel DIAGONAL weight matrices
        D1=diag(invc), D2=diag(-aK*invc); ScalarE drains PSUM -> fp16 out)
The scan keeps fp32 state internally regardless of operand dtype, so fp16
intermediates only add ~3e-4 per-element rounding, no accumulation.

Engine budget per core (8 tiles): DVE ~72us (scan is 2 cyc/elem, its hard
floor), PE ~40us, ACT ~35us, DMA in 16.8MB f32 + out 8.4MB fp16 ~59us.
GpSimd is deliberately idle: its 2-input SBUF ops hold the shared DVE/GpSimd
SBUF port for their whole duration and stall concurrent DVE work.

Sharding: batch dim across 8 cores (4 batches/core -> 1024 channels/core),
no cross-device communication. alpha^K and the reciprocal-weight tables are
precomputed on host from the tiny [F] parameter.
"""

import math
from contextlib import ExitStack

import numpy as np

import concourse.bass as bass
import concourse.mybir as mybir
import concourse.tile as tile
from concourse import bacc
from concourse.bass_utils import run_bass_kernel_spmd

B, F, S = 32, 256, 4096
MAX_SIZE = 200
K = MAX_SIZE + 1          # 201 taps
N_CORES = 8
B_LOC = B // N_CORES      # 4 batches per core
C = B_LOC * F             # 1024 channels per core
P = 128
NT = C // P               # 8 partition tiles per core
NPAR = F // P             # 2 distinct parameter column groups (tile parity)
NS = S - MAX_SIZE         # steady-region columns (3896)
NCHUNK = 8
CW = NS // NCHUNK         # 487 columns per PSUM chunk

F32 = mybir.dt.float32
F16 = mybir.dt.float16
OP_MULT = mybir.AluOpType.mult
OP_ADD = mybir.AluOpType.add


def build_bass():
    nc = bacc.Bacc("TRN2", target_bir_lowering=False, debug=False, num_devices=N_CORES)

    x = nc.declare_dram_parameter("x", [C, S], F32, isOutput=False)
    alpha2 = nc.declare_dram_parameter("alpha2", [P, NPAR], F32, isOutput=False)
    d1m = nc.declare_dram_parameter("d1m", [P, NPAR * P], F16, isOutput=False)
    d2m = nc.declare_dram_parameter("d2m", [P, NPAR * P], F16, isOutput=False)
    invt2 = nc.declare_dram_parameter("invt2", [P, NPAR * MAX_SIZE], F32, isOutput=False)
    out = nc.declare_dram_parameter("out", [C, S], F16, isOutput=True)

    with ExitStack() as ctx:
        tc = ctx.enter_context(tile.TileContext(nc))
        const_pool = ctx.enter_context(tc.tile_pool(name="const", bufs=1))
        xpool = ctx.enter_context(tc.tile_pool(name="xp", bufs=3))
        ypool = ctx.enter_context(tc.tile_pool(name="yp", bufs=3))
        opool = ctx.enter_context(tc.tile_pool(name="op", bufs=3))
        pspool = ctx.enter_context(tc.tile_pool(name="ps", bufs=4, space="PSUM"))

        alpha_sb = const_pool.tile([P, NPAR], F32)
        nc.sync.dma_start(alpha_sb[:], alpha2[:])
        d1_sb = const_pool.tile([P, NPAR * P], F16)
        nc.sync.dma_start(d1_sb[:], d1m[:])
        d2_sb = const_pool.tile([P, NPAR * P], F16)
        nc.sync.dma_start(d2_sb[:], d2m[:])
        invt_sb = const_pool.tile([P, NPAR * MAX_SIZE], F32)
        nc.sync.dma_start(invt_sb[:], invt2[:])

        for j in range(NT):
            p = j % NPAR
            xt = xpool.tile([P, S], F32)
            nc.sync.dma_start(xt[:], x[j * P:(j + 1) * P, :])

            # yf[t] at column t+1; column 0 stays zero (= yf[-1]) so the
            # shifted matmul reads cover t=200 and stay 4B-aligned.
            yft = ypool.tile([P, S + 1], F16)
            nc.gpsimd._memset_packed(yft[:, 0:1], 0)
            nc.vector.tensor_tensor_scan(
                out=yft[:, 1:S + 1],
                data0=alpha_sb[:, p:p + 1].broadcast_to([P, S]),
                data1=xt[:],
                initial=0.0,
                op0=OP_MULT,
                op1=OP_ADD,
            )

            ot = opool.tile([P, S], F16)
            # ramp region t in [0, 200): out = yf[t] * invt[t]
            nc.vector.tensor_mul(
                ot[:, 0:MAX_SIZE],
                yft[:, 1:MAX_SIZE + 1],
                invt_sb[:, p * MAX_SIZE:(p + 1) * MAX_SIZE],
            )
            # steady region t in [200, S) in chunks:
            #   psum = D1^T @ yf[t] ; psum += D2^T @ yf[t-K] ; out = psum
            for c in range(NCHUNK):
                c0 = MAX_SIZE + c * CW
                ps = pspool.tile([P, CW], F32)
                nc.tensor.matmul(
                    ps[:], d1_sb[:, p * P:(p + 1) * P],
                    yft[:, c0 + 1:c0 + 1 + CW],
                    start=True, stop=False,
                )
                nc.tensor.matmul(
                    ps[:], d2_sb[:, p * P:(p + 1) * P],
                    yft[:, c0 - MAX_SIZE:c0 - MAX_SIZE + CW],
                    start=False, stop=True,
                )
                nc.scalar.copy(ot[:, c0:c0 + CW], ps[:])
            nc.sync.dma_start(out[j * P:(j + 1) * P, :], ot[:])

    nc.finalize()
    return nc


_NC_CACHE = None


def _get_nc():
    global _NC_CACHE
    if _NC_CACHE is None:
        _NC_CACHE = build_bass()
    return _NC_CACHE


def _host_params(log_halflife):
    """Precompute per-feature scan/normalization constants (float64 host math)."""
    lh = log_halflife.astype(np.float64)
    alpha = 0.5 ** (1.0 / np.exp(lh))                     # [F]
    aK = alpha ** K                                        # [F]
    powers = alpha[:, None] ** np.arange(K, dtype=np.float64)[None, :]
    csum = np.cumsum(powers, axis=1)                       # [F, K]
    inv_all = 1.0 / (csum + 1e-8)                          # [F, K]
    invc = inv_all[:, MAX_SIZE]                            # [F]

    def fold(v):  # [F, ...] -> [P, NPAR, ...] column-grouped, f = 128*p + lane
        return np.ascontiguousarray(
            v.reshape(NPAR, P, *v.shape[1:]).swapaxes(0, 1)
        )

    alpha2 = fold(alpha).astype(np.float32)                       # [P, NPAR]
    invt2 = (
        fold(inv_all[:, :MAX_SIZE]).reshape(P, NPAR * MAX_SIZE).astype(np.float32)
    )                                                             # [P, NPAR*200]
    # diagonal weight matrices per parity group, [P, NPAR*P] fp16
    d1m = np.zeros((P, NPAR, P), np.float16)
    d2m = np.zeros((P, NPAR, P), np.float16)
    idx = np.arange(P)
    for p in range(NPAR):
        d1m[idx, p, idx] = invc[p * P:(p + 1) * P].astype(np.float16)
        d2m[idx, p, idx] = (-aK * invc)[p * P:(p + 1) * P].astype(np.float16)
    return alpha2, d1m.reshape(P, NPAR * P), d2m.reshape(P, NPAR * P), invt2


def run(x, log_halflife, trace=False):
    x = np.ascontiguousarray(np.asarray(x, dtype=np.float32))
    log_halflife = np.asarray(log_halflife, dtype=np.float32)
    assert x.shape == (B, F, S) and log_halflife.shape == (F,)

    alpha2, d1m, d2m, invt2 = _host_params(log_halflife)
    in_maps = []
    for i in range(N_CORES):
        shard = np.ascontiguousarray(
            x[i * B_LOC:(i + 1) * B_LOC].reshape(C, S)
        )
        in_maps.append({
            "x": shard,
            "alpha2": alpha2,
            "d1m": d1m,
            "d2m": d2m,
            "invt2": invt2,
        })

    nc = _get_nc()
    res = run_bass_kernel_spmd(nc, in_maps, core_ids=list(range(N_CORES)), trace=trace)
    full = np.empty((B, F, S), dtype=np.float32)
    for i in range(N_CORES):
        full[i * B_LOC:(i + 1) * B_LOC] = (
            res.results[i]["out"].astype(np.float32).reshape(B_LOC, F, S)
        )
    return full, res.exec_time_ns


def kernel(x, log_halflife):
    out, _ = run(x, log_halflife, trace=False)
    return out


# revision 7
# speedup vs baseline: 1.7107x; 1.3136x over previous
"""AdaptiveEMA Trainium2 kernel (8 NeuronCores, batch-sharded).

Reference, per channel c=(b,f) and time t (K=201, alpha_c from log_halflife[f]):
    out[c,t] = sum_{k=0..K-1, k<=t} alpha_c^k * x[c,t-k] / (csum_c[min(t,K-1)] + 1e-8)

Math used here: with yf = scan(alpha, x) (untruncated EMA) the truncated sum is
exactly y[t] = yf[t] - alpha^K * yf[t-K], so per [128 x 4096] tile:
    yf   = tensor_tensor_scan(alpha, x)            (DVE; fp32 state, fp16 out,
                                                    2 chained 2048-col segments)
    out[t<200]  = yf[t] * invt[t]                  (DVE ramp multiply, 200 cols)
    out[t>=200] = invc*yf[t] - aK*invc*yf[t-K]     (TensorE: two accumulating
        matmuls per 487-col chunk with per-channel DIAGONAL weight matrices
        D1=diag(invc), D2=diag(-aK*invc); ScalarE drains PSUM -> fp16 out)
The scan keeps fp32 state internally regardless of operand dtype; fp16
intermediates/input add ~4e-4 per-element rounding, no accumulation.

Engine budget per core (8 tiles): DVE ~72us (scan is 2 cyc/elem on the DVE,
its hard floor; dtype-independent), PE ~26us, ACT ~35us, DMA in 8.4MB fp16 +
out 8.4MB fp16 ~47us. GpSimd is deliberately idle: its 2-input SBUF ops hold
the shared DVE/GpSimd SBUF port for their whole duration and stall concurrent
DVE work.

Sharding: batch dim across 8 cores (4 batches/core -> 1024 channels/core),
no cross-device communication. alpha^K and the reciprocal-weight tables are
precomputed on host from the tiny [F] parameter.
"""

import math
from contextlib import ExitStack

import numpy as np

import concourse.bass as bass
import concourse.mybir as mybir
import concourse.tile as tile
from concourse import bacc
from concourse.bass_utils import run_bass_kernel_spmd

B, F, S = 32, 256, 4096
MAX_SIZE = 200
K = MAX_SIZE + 1          # 201 taps
N_CORES = 8
B_LOC = B // N_CORES      # 4 batches per core
C = B_LOC * F             # 1024 channels per core
P = 128
NT = C // P               # 8 partition tiles per core
NPAR = F // P             # 2 distinct parameter column groups (tile parity)
NS = S - MAX_SIZE         # steady-region columns (3896)
NCHUNK = 8
CW = NS // NCHUNK         # 487 columns per PSUM chunk
HALF = S // 2             # scan segment length

F32 = mybir.dt.float32
F16 = mybir.dt.float16
OP_MULT = mybir.AluOpType.mult
OP_ADD = mybir.AluOpType.add


def build_bass():
    nc = bacc.Bacc("TRN2", target_bir_lowering=False, debug=False, num_devices=N_CORES)

    x = nc.declare_dram_parameter("x", [C, S], F16, isOutput=False)
    alpha2 = nc.declare_dram_parameter("alpha2", [P, NPAR], F32, isOutput=False)
    d1m = nc.declare_dram_parameter("d1m", [P, NPAR * P], F16, isOutput=False)
    d2m = nc.declare_dram_parameter("d2m", [P, NPAR * P], F16, isOutput=False)
    invt2 = nc.declare_dram_parameter("invt2", [P, NPAR * MAX_SIZE], F32, isOutput=False)
    out = nc.declare_dram_parameter("out", [C, S], F16, isOutput=True)

    with ExitStack() as ctx:
        tc = ctx.enter_context(tile.TileContext(nc))
        const_pool = ctx.enter_context(tc.tile_pool(name="const", bufs=1))
        xpool = ctx.enter_context(tc.tile_pool(name="xp", bufs=4))
        ypool = ctx.enter_context(tc.tile_pool(name="yp", bufs=3))
        opool = ctx.enter_context(tc.tile_pool(name="op", bufs=3))
        pspool = ctx.enter_context(tc.tile_pool(name="ps", bufs=4, space="PSUM"))

        # issue const loads from the ACT HWDGE ring so the first x-tile DMA
        # is at the head of the Sync ring
        alpha_sb = const_pool.tile([P, NPAR], F32)
        nc.scalar.dma_start(alpha_sb[:], alpha2[:])
        d1_sb = const_pool.tile([P, NPAR * P], F16)
        nc.scalar.dma_start(d1_sb[:], d1m[:])
        d2_sb = const_pool.tile([P, NPAR * P], F16)
        nc.scalar.dma_start(d2_sb[:], d2m[:])
        invt_sb = const_pool.tile([P, NPAR * MAX_SIZE], F32)
        nc.scalar.dma_start(invt_sb[:], invt2[:])

        for j in range(NT):
            p = j % NPAR
            rows = slice(j * P, (j + 1) * P)
            xt = xpool.tile([P, S], F16)
            nc.sync.dma_start(xt[:, 0:HALF], x[rows, 0:HALF])
            nc.sync.dma_start(xt[:, HALF:S], x[rows, HALF:S])

            # yf[t] at column t+1; column 0 stays zero (= yf[-1]) so the
            # shifted matmul reads cover t=200 and stay 4B-aligned.
            yft = ypool.tile([P, S + 1], F16)
            nc.gpsimd._memset_packed(yft[:, 0:1], 0)
            nc.vector.tensor_tensor_scan(
                out=yft[:, 1:1 + HALF],
                data0=alpha_sb[:, p:p + 1].broadcast_to([P, HALF]),
                data1=xt[:, 0:HALF],
                initial=0.0,
                op0=OP_MULT,
                op1=OP_ADD,
            )
            nc.vector.tensor_tensor_scan(
                out=yft[:, 1 + HALF:1 + S],
                data0=alpha_sb[:, p:p + 1].broadcast_to([P, S - HALF]),
                data1=xt[:, HALF:S],
                initial=yft[:, HALF:HALF + 1],
                op0=OP_MULT,
                op1=OP_ADD,
            )

            ot = opool.tile([P, S], F16)
            # ramp region t in [0, 200): out = yf[t] * invt[t]
            nc.vector.tensor_mul(
                ot[:, 0:MAX_SIZE],
                yft[:, 1:MAX_SIZE + 1],
                invt_sb[:, p * MAX_SIZE:(p + 1) * MAX_SIZE],
            )
            # steady region t in [200, S) in chunks:
            #   psum = D1^T @ yf[t] ; psum += D2^T @ yf[t-K] ; out = psum
            for c in range(NCHUNK):
                c0 = MAX_SIZE + c * CW
                ps = pspool.tile([P, CW], F32)
                nc.tensor.matmul(
                    ps[:], d1_sb[:, p * P:(p + 1) * P],
                    yft[:, c0 + 1:c0 + 1 + CW],
                    start=True, stop=False,
                )
                nc.tensor.matmul(
                    ps[:], d2_sb[:, p * P:(p + 1) * P],
                    yft[:, c0 - MAX_SIZE:c0 - MAX_SIZE + CW],
                    start=False, stop=True,
                )
                nc.scalar.copy(ot[:, c0:c0 + CW], ps[:])
            nc.scalar.dma_start(out[rows, :], ot[:])

    nc.finalize()
    return nc


_NC_CACHE = None


def _get_nc():
    global _NC_CACHE
    if _NC_CACHE is None:
        _NC_CACHE = build_bass()
    return _NC_CACHE


def _host_params(log_halflife):
    """Precompute per-feature scan/normalization constants (float64 host math)."""
    lh = log_halflife.astype(np.float64)
    alpha = 0.5 ** (1.0 / np.exp(lh))                     # [F]
    aK = alpha ** K                                        # [F]
    powers = alpha[:, None] ** np.arange(K, dtype=np.float64)[None, :]
    csum = np.cumsum(powers, axis=1)                       # [F, K]
    inv_all = 1.0 / (csum + 1e-8)                          # [F, K]
    invc = inv_all[:, MAX_SIZE]                            # [F]

    def fold(v):  # [F, ...] -> [P, NPAR, ...] column-grouped, f = 128*p + lane
        return np.ascontiguousarray(
            v.reshape(NPAR, P, *v.shape[1:]).swapaxes(0, 1)
        )

    alpha2 = fold(alpha).astype(np.float32)                       # [P, NPAR]
    invt2 = (
        fold(inv_all[:, :MAX_SIZE]).reshape(P, NPAR * MAX_SIZE).astype(np.float32)
    )                                                             # [P, NPAR*200]
    # diagonal weight matrices per parity group, [P, NPAR*P] fp16
    d1m = np.zeros((P, NPAR, P), np.float16)
    d2m = np.zeros((P, NPAR, P), np.float16)
    idx = np.arange(P)
    for p in range(NPAR):
        d1m[idx, p, idx] = invc[p * P:(p + 1) * P].astype(np.float16)
        d2m[idx, p, idx] = (-aK * invc)[p * P:(p + 1) * P].astype(np.float16)
    return alpha2, d1m.reshape(P, NPAR * P), d2m.reshape(P, NPAR * P), invt2


def run(x, log_halflife, trace=False):
    x = np.asarray(x)
    log_halflife = np.asarray(log_halflife, dtype=np.float32)
    assert x.shape == (B, F, S) and log_halflife.shape == (F,)

    alpha2, d1m, d2m, invt2 = _host_params(log_halflife)
    x16 = x.astype(np.float16)
    in_maps = []
    for i in range(N_CORES):
        shard = np.ascontiguousarray(
            x16[i * B_LOC:(i + 1) * B_LOC].reshape(C, S)
        )
        in_maps.append({
            "x": shard,
            "alpha2": alpha2,
            "d1m": d1m,
            "d2m": d2m,
            "invt2": invt2,
        })

    nc = _get_nc()
    res = run_bass_kernel_spmd(nc, in_maps, core_ids=list(range(N_CORES)), trace=trace)
    full = np.empty((B, F, S), dtype=np.float32)
    for i in range(N_CORES):
        full[i * B_LOC:(i + 1) * B_LOC] = (
            res.results[i]["out"].astype(np.float32).reshape(B_LOC, F, S)
        )
    return full, res.exec_time_ns


def kernel(x, log_halflife):
    out, _ = run(x, log_halflife, trace=False)
    return out
